# revision 2
# baseline (speedup 1.0000x reference)
"""Distributed linear (ROW_PARALLEL) on 8 TRN2 NeuronCores.

out = (x.fp16 @ weight.fp16.T).fp32 + bias          x:[8192,4096] w:[16384,4096]

Sharding: tensor-parallel over out_features — core i computes the
[8192, 2048] slab out[:, i*2048:(i+1)*2048]; host concatenates.

Per-core device kernel (weight-stationary, LDW-amortized):
  - w shard [4096, 2048] fp16 = 16 MB resident in SBUF, ko-major layout,
    DMA'd in 8 chunks so compute starts after the first 2 MB
  - x streamed per 128-row m-tile; one LDWEIGHTS (x k-subtile) feeds 4
    matmuls (one per 512-wide n-tile, 4 concurrent psum banks)
  - psum [128, 512] fp32 accumulates 32 k-matmuls (K=4096 = 32 x 128)
  - bias added in fp32 during the psum->sbuf eviction (vector engine)

Host pre-arranges fp16 operands so every DMA is per-partition contiguous.
"""

import numpy as np

import concourse.mybir as mybir
import concourse.tile as tile
from concourse import bacc
from concourse.bass import ts
from concourse.bass_utils import run_bass_kernel_spmd

M, K, N = 8192, 4096, 16384
NCORES = 8
NSH = N // NCORES       # 2048 out-features per core
P = 128
KO = K // P             # 32 k-subtiles
MT = M // P             # 64 m-tiles
NFREE = 512             # psum free dim (one bank, fp32)
NT = NSH // NFREE       # 4 n-tiles per core
W_CHUNK = 4             # ko per w-load DMA chunk

_cached = None


def _build(reps=1):
    nc = bacc.Bacc("TRN2", target_bir_lowering=False, debug=False,
                   num_devices=NCORES)
    xt = nc.dram_tensor("xt", [MT, P, KO, P], mybir.dt.float16,
                        kind="ExternalInput")
    wt = nc.dram_tensor("wt", [P, KO, NT, NFREE], mybir.dt.float16,
                        kind="ExternalInput")
    bb = nc.dram_tensor("bb", [P, NSH], mybir.dt.float32,
                        kind="ExternalInput")
    out = nc.dram_tensor("out", [MT, P, NSH], mybir.dt.float32,
                         kind="ExternalOutput")

    with tile.TileContext(nc) as tc:
        with (
            tc.tile_pool(name="wpool", bufs=1) as wpool,
            tc.tile_pool(name="xpool", bufs=3) as xpool,
            tc.tile_pool(name="opool", bufs=3) as opool,
            tc.tile_pool(name="cpool", bufs=1) as cpool,
            tc.tile_pool(name="pspool", bufs=2, space="PSUM") as pspool,
        ):
            def body():
                w_sb = wpool.tile([P, KO, NT, NFREE], mybir.dt.float16)
                for kc in range(0, KO, W_CHUNK):
                    nc.sync.dma_start(
                        w_sb[:, kc:kc + W_CHUNK], wt[:, kc:kc + W_CHUNK]
                    )
                bias_sb = cpool.tile([P, NSH], mybir.dt.float32)
                nc.sync.dma_start(bias_sb[:], bb[:])

                for mt in range(MT):
                    x_sb = xpool.tile([P, KO, P], mybir.dt.float16)
                    nc.sync.dma_start(x_sb[:], xt[mt])
                    o_sb = opool.tile([P, NSH], mybir.dt.float32)
                    # one LDW (x k-subtile) feeds NT matmuls, NT psum banks
                    pss = [
                        pspool.tile([P, NFREE], mybir.dt.float32,
                                    tag=f"ps{nt}", name=f"ps{nt}")
                        for nt in range(NT)
                    ]
                    for ko in range(KO):
                        for nt in range(NT):
                            nc.tensor.matmul(
                                pss[nt][:], x_sb[:, ko], w_sb[:, ko, nt],
                                start=(ko == 0), stop=(ko == KO - 1),
                            )
                    for nt in range(NT):
                        nc.vector.tensor_add(
                            o_sb[:, ts(nt, NFREE)], pss[nt][:],
                            bias_sb[:, ts(nt, NFREE)],
                        )
                    nc.sync.dma_start(out[mt], o_sb[:])

            if reps == 1:
                body()
            else:
                with tc.For_i(0, reps):
                    body()

    nc.compile()
    return nc


def _get_nc():
    global _cached
    if _cached is None:
        _cached = _build()
    return _cached


def prep_in_maps(x: np.ndarray, weight: np.ndarray, bias: np.ndarray,
                 dt16=np.float16):
    x16 = np.asarray(x, dtype=dt16)
    w16 = np.asarray(weight, dtype=dt16)
    b32 = np.asarray(bias, dtype=np.float32)

    # xt[mt, p, ko, m] = x16[mt*128 + m, ko*128 + p]  (replicated to all cores)
    xt = np.ascontiguousarray(
        x16.reshape(MT, P, KO, P).transpose(0, 3, 2, 1)
    )

    in_maps = []
    for i in range(NCORES):
        wsh = w16[i * NSH:(i + 1) * NSH]              # [2048, 4096]
        # wt[p, ko, nt, nf] = wsh[nt*512 + nf, ko*128 + p]
        wti = np.ascontiguousarray(
            wsh.reshape(NT, NFREE, KO, P).transpose(3, 2, 0, 1)
        )
        bsh = np.ascontiguousarray(
            np.broadcast_to(b32[i * NSH:(i + 1) * NSH], (P, NSH))
        )
        in_maps.append({"xt": xt, "wt": wti, "bb": bsh})
    return in_maps


def kernel(x: np.ndarray, weight: np.ndarray, bias: np.ndarray) -> np.ndarray:
    in_maps = prep_in_maps(x, weight, bias)
    nc = _get_nc()
    res = run_bass_kernel_spmd(nc, in_maps, core_ids=list(range(NCORES)))
    shards = [res.results[i]["out"].reshape(M, NSH) for i in range(NCORES)]
    return np.concatenate(shards, axis=1)



# revision 5
# speedup vs baseline: 1.0133x; 1.0133x over previous
"""Distributed linear (ROW_PARALLEL) on 8 TRN2 NeuronCores.

out = (x.fp16 @ weight.fp16.T).fp32 + bias          x:[8192,4096] w:[16384,4096]

Sharding: tensor-parallel over out_features — core i computes the
[8192, 2048] slab out[:, i*2048:(i+1)*2048]; host concatenates.

Per-core device kernel (weight-stationary, LDW-amortized):
  - w shard [4096, 2048] fp16 = 16 MB resident in SBUF, ko-major layout,
    DMA'd in chunks on the scalar-engine DGE queue so the x stream (SP
    queue) isn't blocked behind it at startup
  - x streamed per 128-row m-tile; one LDWEIGHTS (x k-subtile) feeds 4
    matmuls (one per 512-wide n-tile, 4 concurrent psum banks); the 3
    redundant LDWEIGHTS bass emits per matmul group are deleted post-
    scheduling (hardware keeps the stationary operand loaded)
  - psum [128, 512] fp32 accumulates 32 k-matmuls (K=4096 = 32 x 128)
  - bias added in fp32 during the psum->sbuf eviction (vector engine)

Host pre-arranges fp16 operands so every DMA is per-partition contiguous.
"""

import numpy as np

import concourse.mybir as mybir
import concourse.tile as tile
from concourse import bacc
from concourse.bass import ts
from concourse.bass_utils import run_bass_kernel_spmd

M, K, N = 8192, 4096, 16384
NCORES = 8
NSH = N // NCORES       # 2048 out-features per core
P = 128
KO = K // P             # 32 k-subtiles
MT = M // P             # 64 m-tiles
NFREE = 512             # psum free dim (one bank, fp32)
NT = NSH // NFREE       # 4 n-tiles per core
W_CHUNK = 2             # ko per w-load DMA chunk

_cached = None


def _strip_redundant_ldweights(nc):
    """Remove InstLdweights that reload the stationary AP already held by the
    PE array (bass emits one LDW per matmul; hardware keeps the loaded
    operand across matmuls). A reload is redundant when the previous LDW in
    the block has an identical AP and the source buffer was not rewritten in
    between. Dependency edges referencing a removed LDW are remapped to the
    matmul that follows it."""
    PE = mybir.EngineType.PE
    fn = nc.m.functions[0]
    removed_map = {}  # removed ldw name -> following matmul name
    for bb in fn.blocks:
        insts = bb.instructions
        cur_key = None
        cur_memref = None
        pending = []   # indices of redundant LDWs awaiting their next MM
        to_remove = []
        for k, inst in enumerate(insts):
            t = type(inst).__name__
            eng = getattr(inst, "engine", None)
            if eng != PE:
                # anything that writes the loaded buffer invalidates reuse
                if cur_memref is not None and any(
                    getattr(o, "memref", None) == cur_memref
                    for o in getattr(inst, "outs", [])
                ):
                    cur_key = None
                    cur_memref = None
                continue
            if t == "InstLdweights":
                key = str(inst.ins[0])
                if key == cur_key:
                    pending.append(k)
                    to_remove.append(k)
                else:
                    cur_key = key
                    cur_memref = getattr(inst.ins[0], "memref", None)
            elif t == "InstMatmult":
                for j in pending:
                    removed_map[insts[j].name] = inst.name
                pending = []
            elif t == "InstEventSemaphore":
                pass
            else:
                cur_key = None
                cur_memref = None
        assert not pending, "redundant LDW with no following matmul"
        for k in reversed(to_remove):
            del insts[k]
    if removed_map:
        for bb in fn.blocks:
            for inst in bb.instructions:
                deps = set(inst.sync_dependency_names()) | set(
                    inst.nosync_dependency_names())
                if deps & set(removed_map):
                    inst.remap_dependency_names(removed_map)
    return len(removed_map)


def _build(reps=1):
    nc = bacc.Bacc("TRN2", target_bir_lowering=False, debug=False,
                   num_devices=NCORES)
    xt = nc.dram_tensor("xt", [MT, P, KO, P], mybir.dt.float16,
                        kind="ExternalInput")
    wt = nc.dram_tensor("wt", [P, KO, NT, NFREE], mybir.dt.float16,
                        kind="ExternalInput")
    bb = nc.dram_tensor("bb", [P, NSH], mybir.dt.float32,
                        kind="ExternalInput")
    out = nc.dram_tensor("out", [MT, P, NSH], mybir.dt.float32,
                         kind="ExternalOutput")

    with tile.TileContext(nc) as tc:
        with (
            tc.tile_pool(name="wpool", bufs=1) as wpool,
            tc.tile_pool(name="xpool", bufs=3) as xpool,
            tc.tile_pool(name="opool", bufs=3) as opool,
            tc.tile_pool(name="cpool", bufs=1) as cpool,
            tc.tile_pool(name="pspool", bufs=2, space="PSUM") as pspool,
        ):
            def body():
                # w + bias on the scalar-engine DGE queue so the x stream
                # (SP queue) is not blocked behind 16 MB of weights
                w_sb = wpool.tile([P, KO, NT, NFREE], mybir.dt.float16)
                for kc in range(0, KO, W_CHUNK):
                    nc.scalar.dma_start(
                        w_sb[:, kc:kc + W_CHUNK], wt[:, kc:kc + W_CHUNK]
                    )
                bias_sb = cpool.tile([P, NSH], mybir.dt.float32)
                nc.scalar.dma_start(bias_sb[:], bb[:])

                for mt in range(MT):
                    x_sb = xpool.tile([P, KO, P], mybir.dt.float16)
                    nc.sync.dma_start(x_sb[:], xt[mt])
                    o_sb = opool.tile([P, NSH], mybir.dt.float32)
                    # one LDW (x k-subtile) feeds NT matmuls, NT psum banks
                    pss = [
                        pspool.tile([P, NFREE], mybir.dt.float32,
                                    tag=f"ps{nt}", name=f"ps{nt}")
                        for nt in range(NT)
                    ]
                    for ko in range(KO):
                        for nt in range(NT):
                            nc.tensor.matmul(
                                pss[nt][:], x_sb[:, ko], w_sb[:, ko, nt],
                                start=(ko == 0), stop=(ko == KO - 1),
                            )
                    for nt in range(NT):
                        nc.vector.tensor_add(
                            o_sb[:, ts(nt, NFREE)], pss[nt][:],
                            bias_sb[:, ts(nt, NFREE)],
                        )
                    nc.sync.dma_start(out[mt], o_sb[:])

            if reps == 1:
                body()
            else:
                with tc.For_i(0, reps):
                    body()

    _strip_redundant_ldweights(nc)
    nc.compile()
    return nc


def _get_nc():
    global _cached
    if _cached is None:
        _cached = _build()
    return _cached


def prep_in_maps(x: np.ndarray, weight: np.ndarray, bias: np.ndarray,
                 dt16=np.float16):
    x16 = np.asarray(x, dtype=dt16)
    w16 = np.asarray(weight, dtype=dt16)
    b32 = np.asarray(bias, dtype=np.float32)

    # xt[mt, p, ko, m] = x16[mt*128 + m, ko*128 + p]  (replicated to all cores)
    xt = np.ascontiguousarray(
        x16.reshape(MT, P, KO, P).transpose(0, 3, 2, 1)
    )

    in_maps = []
    for i in range(NCORES):
        wsh = w16[i * NSH:(i + 1) * NSH]              # [2048, 4096]
        # wt[p, ko, nt, nf] = wsh[nt*512 + nf, ko*128 + p]
        wti = np.ascontiguousarray(
            wsh.reshape(NT, NFREE, KO, P).transpose(3, 2, 0, 1)
        )
        bsh = np.ascontiguousarray(
            np.broadcast_to(b32[i * NSH:(i + 1) * NSH], (P, NSH))
        )
        in_maps.append({"xt": xt, "wt": wti, "bb": bsh})
    return in_maps


def kernel(x: np.ndarray, weight: np.ndarray, bias: np.ndarray) -> np.ndarray:
    in_maps = prep_in_maps(x, weight, bias)
    nc = _get_nc()
    res = run_bass_kernel_spmd(nc, in_maps, core_ids=list(range(NCORES)))
    shards = [res.results[i]["out"].reshape(M, NSH) for i in range(NCORES)]
    return np.concatenate(shards, axis=1)
